# revision 32
# baseline (speedup 1.0000x reference)
"""2-layer GCN (GCNConv x2, relu) on 8 Trainium2 NeuronCores.

Strategy (dest-shard, aggregate in 128-dim space, host-built one-hots):
  out1 = relu((A1*x) @ W1 + b1)         [A@(x@W1) == (A@x)@W1]
  z    = out1 @ W2 -> zg, allgathered
  out2 = relu(A1*zg + b2)
where A1 = Dc*(Mw + I)*Dr with the FULL normalization norm_e =
dinv[row]*w*dinv[col] folded into host-precomputed one-hot tiles
oh[e, d] = norm_e * (d == localdest_e), shared by both layers.

Per core: 1/8 of destination nodes. Edges sorted by dest panel (128
dests), chunked into groups of 128. L1 sources are HOST-pregathered
into a sequential stream xg (no device gather). L2 sources are
dma_gather'ed from the allgathered zg (int16 idx, split in 2 halves
< 32768 rows). Per chunk one matmul accumulates into the panel PSUM.
"""
import sys
import numpy as np

sys.path.insert(0, "/opt/trn_rl_repo")

import concourse.bass as bass  # noqa: F401
import concourse.bacc as bacc
import concourse.mybir as mybir
import concourse.tile as tile
from concourse.bass_utils import run_bass_kernel_spmd
from concourse.masks import make_identity

P = 128
NCORES = 8
G = 16    # chunks per dma_gather wave (L2)
SW = 32     # chunks per xg stream DMA wave
SW_OH = 8  # chunks per L2 one-hot stream DMA wave
LS_G = 8  # chunks per local_scatter one-hot build (L1)

F32 = mybir.dt.float32
F16 = mybir.dt.float16
I16 = mybir.dt.int16
MSG_DT = F16  # dtype for messages / dense matmuls
NP_MSG = np.float16


# ---------------------------------------------------------------- CPU prep


def _pack_idx(idx_flat):
    """int16 indices -> [128, ceil(n/16)] wrapped + 8x replicated layout."""
    n = len(idx_flat)
    n16 = -(-n // 16)
    buf = np.zeros(16 * n16, dtype=np.int16)
    buf[:n] = idx_flat
    blk = buf.reshape(n16, 16).T  # idx j at [j%16, j//16]
    return np.tile(blk, (8, 1)).copy()


def preprocess(x, edge_index, edge_weight, n):
    """Per-core inputs + universal chunk grid.

    Dests are grouped into panels of <=128 CONSECUTIVE dests with
    adaptive boundaries chosen per core to fill chunk capacities; the
    per-panel chunk counts (klo_j, khi_j) are universal (max over
    cores). Outputs and zg are written panel-padded (128 rows/panel);
    layer-2 gather indices use AG positions in the padded layout.
    """
    row = np.asarray(edge_index[0], dtype=np.int64)
    col = np.asarray(edge_index[1], dtype=np.int64)
    w = np.asarray(edge_weight, dtype=np.float32)
    shard = n // NCORES
    half = (n + 1) // 2

    # ---- full normalization on host: deg incl self loops, norm per edge
    deg = np.zeros(n, np.float64)
    np.add.at(deg, col, w.astype(np.float64))
    deg += 1.0  # self loop w=1
    dinv = 1.0 / np.sqrt(deg)
    norm = (dinv[row] * w * dinv[col]).astype(np.float32)

    core_of = col // shard

    # ---- per-dest lo/hi in-degree (incl self edge) per core
    # Stream split = source's PANEL-half (enables split AllGather); for
    # boundary construction approximate it by shard-local half.
    hs = shard // 2
    deg_lo = np.zeros(n, np.int64)
    deg_hi = np.zeros(n, np.int64)
    np.add.at(deg_lo, col[(row % shard) < hs], 1)
    np.add.at(deg_hi, col[(row % shard) >= hs], 1)
    selfs_all = np.arange(n)
    deg_lo[(selfs_all % shard) < hs] += 1
    deg_hi[(selfs_all % shard) >= hs] += 1

    # ---- adaptive consecutive panel boundaries per core
    CAP = 7 * P  # max edges per (panel, stream)
    blist = []
    for k in range(NCORES):
        lo_c = deg_lo[k * shard:(k + 1) * shard]
        hi_c = deg_hi[k * shard:(k + 1) * shard]
        b = [0]
        cl = ch = cd = 0
        for ld in range(shard):
            if cd == P or cl + lo_c[ld] > CAP or ch + hi_c[ld] > CAP:
                b.append(ld)
                cl = ch = cd = 0
            cl += lo_c[ld]
            ch += hi_c[ld]
            cd += 1
        b.append(shard)
        blist.append(b)
    npanel = max(len(b) - 1 for b in blist)
    bounds = np.zeros((NCORES, npanel + 1), np.int64)
    for k in range(NCORES):
        b = blist[k]
        while len(b) < npanel + 1:
            b.append(shard)
        bounds[k] = b

    # ---- AG position map (half-major: all cores' first-j0 panels, then
    # the rest) + EXACT stream split by source's panel-half
    j0 = npanel // 2
    off1 = NCORES * j0 * P
    agpos = np.zeros(n, np.int64)
    for k in range(NCORES):
        for j in range(npanel):
            a, b2 = bounds[k, j], bounds[k, j + 1]
            if b2 > a:
                if j < j0:
                    base = k * j0 * P + j * P
                else:
                    base = off1 + k * (npanel - j0) * P + (j - j0) * P
                agpos[k * shard + a:k * shard + b2] = \
                    base + np.arange(b2 - a)
    hi_src = agpos >= off1
    h2 = off1
    assert h2 < 32768 and (NCORES * npanel * P - h2) < 32768

    # exact per-dest stream in-degrees for the universal grid
    deg_lo = np.zeros(n, np.int64)
    deg_hi = np.zeros(n, np.int64)
    np.add.at(deg_lo, col[~hi_src[row]], 1)
    np.add.at(deg_hi, col[hi_src[row]], 1)
    deg_lo[~hi_src] += 1  # self loops
    deg_hi[hi_src] += 1

    # ---- per (core, panel, stream) edge counts -> universal grid
    cnts = np.zeros((NCORES, npanel, 2), np.int64)
    for k in range(NCORES):
        for j in range(npanel):
            a, b2 = bounds[k, j], bounds[k, j + 1]
            cnts[k, j, 0] = deg_lo[k * shard + a:k * shard + b2].sum()
            cnts[k, j, 1] = deg_hi[k * shard + a:k * shard + b2].sum()
    kcnt = np.maximum(1, -(-cnts.max(axis=0) // P))  # [npanel, 2]
    klo, khi = kcnt[:, 0], kcnt[:, 1]
    nlo_ch, nhi_ch = int(klo.sum()), int(khi.sum())
    nchunks = nlo_ch + nhi_ch

    chunks = []
    lo_pos = hi_pos = 0
    for j in range(npanel):
        nj = int(klo[j]) + int(khi[j])
        for i in range(int(klo[j])):
            chunks.append(dict(stream=0, pos=lo_pos, panel=j,
                               first=(i == 0), last=(i == nj - 1)))
            lo_pos += 1
        for i in range(int(khi[j])):
            chunks.append(dict(stream=1, pos=hi_pos, panel=j,
                               first=False, last=(int(klo[j]) + i == nj - 1)))
            hi_pos += 1

    x16 = np.ascontiguousarray(np.asarray(x, dtype=NP_MSG))

    # ---- per-core chunk slot data
    cores = []
    for k in range(NCORES):
        m = core_of == k
        selfs = np.arange(k * shard, (k + 1) * shard, dtype=np.int64)
        r_k = np.concatenate([row[m], selfs])
        c_k = np.concatenate([col[m], selfs])
        nm_k = np.concatenate([norm[m],
                               (dinv[selfs] * dinv[selfs]).astype(np.float32)])
        ld = c_k - k * shard
        panel = np.searchsorted(bounds[k], ld, side="right") - 1
        q = ld - bounds[k][panel]  # slot within panel
        hi = hi_src[r_k].astype(np.int64)
        order = np.lexsort((hi, panel))
        r_k, nm_k, panel, hi, q = (r_k[order], nm_k[order], panel[order],
                                   hi[order], q[order])
        key = panel * 2 + hi
        cnt = np.bincount(key, minlength=npanel * 2)
        goff = np.r_[0, np.cumsum(cnt)]
        src_mat = np.zeros((nchunks, P), np.int64)     # source node per slot
        idx2_lo = np.zeros(nlo_ch * P, np.int64)
        idx2_hi = np.zeros(nhi_ch * P, np.int64)
        lc_mat = np.zeros((nchunks, P), np.int64)      # local dest per slot
        nm_mat = np.zeros((nchunks, P), np.float32)    # norm per slot
        win = np.zeros(npanel * 2, np.int64)
        ag_r = agpos[r_k]
        for ci, c in enumerate(chunks):
            j, st = c["panel"], c["stream"]
            g = j * 2 + st
            a = goff[g] + win[g] * P
            b2 = min(goff[g] + win[g] * P + P, goff[g + 1])
            win[g] += 1
            m2 = max(0, b2 - a)
            if m2 > 0:
                pos = c["pos"] * P
                if st == 0:
                    idx2_lo[pos:pos + m2] = ag_r[a:b2]
                else:
                    idx2_hi[pos:pos + m2] = ag_r[a:b2] - h2
                src_mat[ci, :m2] = r_k[a:b2]
                lc_mat[ci, :m2] = q[a:b2]
                nm_mat[ci, :m2] = nm_k[a:b2]

        # host-pregathered L1 source stream: [128 e, nchunks, 128 f]
        xg = np.ascontiguousarray(
            x16[src_mat].transpose(1, 0, 2))
        # L2 one-hot tiles streamed from HBM: [128 e, nchunks, 128 d]
        ohm = np.zeros((nchunks, P, P), NP_MSG)
        cii, eii = np.nonzero(nm_mat)
        ohm[cii, eii, lc_mat[cii, eii]] = nm_mat[cii, eii].astype(NP_MSG)
        ohm = np.ascontiguousarray(ohm.transpose(1, 0, 2))
        # compact one-hot descriptors for L1's on-device local_scatter build
        ncpad = -(-nchunks // LS_G) * LS_G
        norm16 = np.zeros((P, ncpad), NP_MSG)
        norm16[:, :nchunks] = nm_mat.T.astype(NP_MSG)
        ci_grid = np.broadcast_to(np.arange(nchunks)[:, None], (nchunks, P))
        lsidx = np.full((P, ncpad), -1, np.int16)
        lsidx[:, :nchunks] = np.where(
            nm_mat != 0, (ci_grid % LS_G) * P + lc_mat, -1).T.astype(np.int16)
        cores.append(dict(idx2_lo=_pack_idx(idx2_lo.astype(np.int16)),
                          idx2_hi=_pack_idx(idx2_hi.astype(np.int16)),
                          xg=xg, ohm=ohm, norm16=norm16, lsidx=lsidx,
                          bounds=bounds[k].copy()))

    spec = dict(n=n, shard=shard, npanel=npanel, half=half, h2=h2, j0=j0,
                chunks=chunks, nlo=nlo_ch, nhi=nhi_ch, nchunks=nchunks,
                ncpad=ncpad)
    return spec, cores


# ---------------------------------------------------------------- program


def build_program(spec, din, dhid, dout, has_b1, has_b2):
    n, half = spec["n"], spec["half"]
    npanel, shard = spec["npanel"], spec["shard"]
    h2, j0 = spec["h2"], spec["j0"]
    nrows_pad = npanel * P  # padded rows per core (zg_shard / out)
    nfull = NCORES * nrows_pad
    chunks, nlo, nhi = spec["chunks"], spec["nlo"], spec["nhi"]
    nchunks = spec["nchunks"]
    assert din == P and dout == P and dhid == 2 * P

    nc = bacc.Bacc("TRN2", target_bir_lowering=False, debug=False,
                   num_devices=NCORES, num_swdge_queues=4)
    dt = F32
    ncpad = spec["ncpad"]
    xg_d = nc.dram_tensor("xg", [P, nchunks, din], MSG_DT,
                          kind="ExternalInput")
    oh_d = nc.dram_tensor("ohm", [P, nchunks, P], MSG_DT,
                          kind="ExternalInput")
    norm16_d = nc.dram_tensor("norm16", [P, ncpad], MSG_DT,
                              kind="ExternalInput")
    lsidx_d = nc.dram_tensor("lsidx", [P, ncpad], I16, kind="ExternalInput")
    w1_d = nc.dram_tensor("w1", [din, dhid], MSG_DT, kind="ExternalInput")
    w2_d = nc.dram_tensor("w2", [dhid, dout], MSG_DT, kind="ExternalInput")
    i2lo_d = nc.dram_tensor("idx2_lo", [P, nlo * 8], I16,
                            kind="ExternalInput")
    i2hi_d = nc.dram_tensor("idx2_hi", [P, nhi * 8], I16,
                            kind="ExternalInput")
    b1_d = b2_d = None
    if has_b1:
        b1_d = nc.dram_tensor("b1r", [P, dhid], dt, kind="ExternalInput")
    if has_b2:
        b2_d = nc.dram_tensor("b2r", [P, dout], dt, kind="ExternalInput")
    out_d = nc.dram_tensor("out", [nrows_pad, dout], dt,
                           kind="ExternalOutput")

    with tile.TileContext(nc) as tc:
        with (
            tc.tile_pool(name="const", bufs=1) as cpool,
            tc.tile_pool(name="dram", bufs=1, space="DRAM") as dram,
        ):
            zg_shard = dram.tile([nrows_pad, dout], MSG_DT)
            zg_lo = dram.tile([h2, dout], MSG_DT, addr_space="Shared")
            zg_hi = dram.tile([nfull - h2, dout], MSG_DT,
                              addr_space="Shared")

            ident = cpool.tile([P, P], MSG_DT, tag="ident")
            make_identity(nc, ident[:])
            w1_sb = cpool.tile([din, dhid], MSG_DT, tag="w1")
            nc.sync.dma_start(out=w1_sb[:], in_=w1_d[:])
            w2a_sb = cpool.tile([P, dout], MSG_DT, tag="w2a")
            nc.sync.dma_start(out=w2a_sb[:], in_=w2_d[0:P, :])
            w2b_sb = cpool.tile([P, dout], MSG_DT, tag="w2b")
            nc.sync.dma_start(out=w2b_sb[:], in_=w2_d[P:2 * P, :])
            i2lo_sb = cpool.tile([P, nlo * 8], I16, tag="i2lo")
            nc.sync.dma_start(out=i2lo_sb[:], in_=i2lo_d[:])
            i2hi_sb = cpool.tile([P, nhi * 8], I16, tag="i2hi")
            nc.sync.dma_start(out=i2hi_sb[:], in_=i2hi_d[:])
            norm16_sb = cpool.tile([P, ncpad], MSG_DT, tag="norm16")
            nc.sync.dma_start(out=norm16_sb[:], in_=norm16_d[:])
            lsidx_sb = cpool.tile([P, ncpad], I16, tag="lsidx")
            nc.sync.dma_start(out=lsidx_sb[:], in_=lsidx_d[:])
            b1_sb = b2_sb = None
            if has_b1:
                b1_sb = cpool.tile([P, dhid], dt, tag="b1")
                nc.sync.dma_start(out=b1_sb[:], in_=b1_d[:])
            if has_b2:
                b2_sb = cpool.tile([P, dout], dt, tag="b2")
                nc.sync.dma_start(out=b2_sb[:], in_=b2_d[:])

            # -------- shared chunk machinery ------------------------------
            def agg_layer(layer, emit_panel, src_lo=None, src_hi=None,
                          ix_lo=None, ix_hi=None, after_wave=None):
                """layer 1: stream xg sequentially; layer 2: dma_gather."""
                with (
                    tc.tile_pool(name=f"glo{layer}", bufs=6) as glo,
                    tc.tile_pool(name=f"ghi{layer}", bufs=6) as ghi,
                    tc.tile_pool(name=f"oh{layer}", bufs=4) as ohp,
                    tc.tile_pool(name=f"agg{layer}", bufs=2 if layer == 1
                                 else 6, space="PSUM") as aggp,
                    tc.tile_pool(name=f"ep{layer}", bufs=1,
                                 space="PSUM") as epp,
                    tc.tile_pool(name=f"sb{layer}", bufs=3) as sbp,
                ):
                    wave_t = [None, None]
                    oh_t = None
                    oh_ls = None
                    psum = None
                    gq = [0]  # round-robin SWDGE queue for gather waves
                    wtiles = [{}, {}]
                    # L1 one-hots: local_scatter only BEFORE AG1 fires so the
                    # GpSimd queue is clear for L2 gathers; stream the rest.
                    ci_sw = next((i for i, cc in enumerate(chunks)
                                  if cc["panel"] >= j0), nchunks)

                    def ensure_wave(st2, wv2):
                        if wv2 in wtiles[st2]:
                            return
                        pool = glo if st2 == 0 else ghi
                        idx_sb = ix_lo if st2 == 0 else ix_hi
                        src = src_lo if st2 == 0 else src_hi
                        nw = nlo if st2 == 0 else nhi
                        gsz = min(G, nw - wv2 * G)
                        t = pool.tile([P, G, P], MSG_DT, tag="gw")
                        nc.gpsimd.dma_gather(
                            out_ap=t[:, :gsz, :], in_ap=src,
                            idxs_ap=idx_sb[:, wv2 * G * 8:
                                           wv2 * G * 8 + gsz * 8],
                            num_idxs=gsz * P, num_idxs_reg=gsz * P,
                            elem_size=P, single_packet=False,
                            queue_num=gq[0])
                        gq[0] = (gq[0] + 1) % 4
                        wtiles[st2][wv2] = t

                    for ci, c in enumerate(chunks):
                        st, pos, j = c["stream"], c["pos"], c["panel"]
                        # one-hot built on device from compact (lc, norm)
                        if layer == 1:  # local_scatter on idle GpSimd
                            wv_o, slot_o = divmod(ci, LS_G)
                            if slot_o == 0:
                                oh_ls = ohp.tile([P, LS_G, P], MSG_DT,
                                                 tag="ohls")
                                a = wv_o * LS_G
                                nc.gpsimd.local_scatter(
                                    out_ap=oh_ls[:],
                                    data_ap=norm16_sb[:, a:a + LS_G],
                                    idxs_ap=lsidx_sb[:, a:a + LS_G],
                                    channels=P, num_elems=LS_G * P,
                                    num_idxs=LS_G)
                            oh = oh_ls[:, slot_o, :]
                        else:           # streamed one-hot wave from HBM
                            base = ci_sw if layer == 1 else 0
                            wv_o, slot_o = divmod(ci - base, SW_OH)
                            if slot_o == 0:
                                a = base + wv_o * SW_OH
                                osz = min(SW_OH, nchunks - a)
                                oh_t = ohp.tile([P, SW_OH, P], MSG_DT,
                                                tag="ohw")
                                nc.sync.dma_start(
                                    out=oh_t[:, :osz, :],
                                    in_=oh_d[:, a:a + osz, :])
                            oh = oh_t[:, slot_o, :]
                        # source tile
                        if layer == 1:
                            gt = None  # xg stream, ci-major
                            wv_x, slot_x = divmod(ci, SW)
                            if slot_x == 0:
                                osz = min(SW, nchunks - wv_x * SW)
                                t = glo.tile([P, SW, P], MSG_DT, tag="xw")
                                eng = nc.scalar if wv_x % 2 else nc.sync
                                eng.dma_start(
                                    out=t[:, :osz, :],
                                    in_=xg_d[:, wv_x * SW:wv_x * SW + osz, :])
                                wave_t[0] = t
                            gt = wave_t[0][:, slot_x, :]
                        else:
                            wv, slot = divmod(pos, G)
                            ensure_wave(st, wv)
                            gt = wtiles[st][wv][:, slot, :]
                        if c["first"]:
                            psum = aggp.tile([P, P], F32, space="PSUM",
                                             tag="agg")
                        if layer == 1:  # feat-major: psum[f,d] += g.T @ oh
                            nc.tensor.matmul(out=psum[:], lhsT=gt, rhs=oh,
                                             start=c["first"], stop=c["last"])
                        else:           # dest-major: psum[d,f] += oh.T @ g
                            nc.tensor.matmul(out=psum[:], lhsT=oh, rhs=gt,
                                             start=c["first"], stop=c["last"])
                        if c["last"]:
                            emit_panel(j, psum, epp, sbp)

            # -------- layer 1 ---------------------------------------------
            def l1_panel(j, psum, epp, sbp):
                aggT = sbp.tile([P, P], MSG_DT, tag="aggT")
                nc.vector.tensor_copy(out=aggT[:], in_=psum[:])
                h1p = epp.tile([P, dhid], F32, space="PSUM", tag="h1p")
                nc.tensor.matmul(out=h1p[:], lhsT=aggT[:], rhs=w1_sb[:],
                                 start=True, stop=True)
                h1 = sbp.tile([P, dhid], MSG_DT, tag="h1")
                if has_b1:
                    nc.vector.tensor_tensor(out=h1[:], in0=h1p[:],
                                            in1=b1_sb[:],
                                            op=mybir.AluOpType.add)
                    nc.vector.tensor_scalar(out=h1[:], in0=h1[:],
                                            scalar1=0.0, scalar2=None,
                                            op0=mybir.AluOpType.max)
                else:
                    nc.vector.tensor_scalar(out=h1[:], in0=h1p[:],
                                            scalar1=0.0, scalar2=None,
                                            op0=mybir.AluOpType.max)
                tp0 = epp.tile([P, P], MSG_DT, space="PSUM", tag="tp0")
                nc.tensor.transpose(out=tp0[:], in_=h1[:, 0:P],
                                    identity=ident[:])
                tp1 = epp.tile([P, P], MSG_DT, space="PSUM", tag="tp1")
                nc.tensor.transpose(out=tp1[:], in_=h1[:, P:2 * P],
                                    identity=ident[:])
                h1t0 = sbp.tile([P, P], MSG_DT, tag="h1t0")
                nc.vector.tensor_copy(out=h1t0[:], in_=tp0[:])
                h1t1 = sbp.tile([P, P], MSG_DT, tag="h1t1")
                nc.vector.tensor_copy(out=h1t1[:], in_=tp1[:])
                zp = epp.tile([P, dout], F32, space="PSUM", tag="zp")
                nc.tensor.matmul(out=zp[:], lhsT=h1t0[:], rhs=w2a_sb[:],
                                 start=True, stop=False)
                nc.tensor.matmul(out=zp[:], lhsT=h1t1[:], rhs=w2b_sb[:],
                                 start=False, stop=True)
                zg = sbp.tile([P, dout], MSG_DT, tag="zg")
                nc.vector.tensor_copy(out=zg[:], in_=zp[:])
                nc.sync.dma_start(out=zg_shard[j * P:(j + 1) * P, :],
                                  in_=zg[:])
                if j == j0 - 1:
                    # first-half AllGather overlaps the rest of layer 1
                    nc.gpsimd.collective_compute(
                        "AllGather", mybir.AluOpType.bypass,
                        replica_groups=[list(range(NCORES))],
                        ins=[zg_shard[0:j0 * P, :].opt()],
                        outs=[zg_lo[:].opt()])

            agg_layer(1, l1_panel)

            nc.gpsimd.collective_compute(
                "AllGather", mybir.AluOpType.bypass,
                replica_groups=[list(range(NCORES))],
                ins=[zg_shard[j0 * P:nrows_pad, :].opt()],
                outs=[zg_hi[:].opt()])

            # -------- layer 2 ---------------------------------------------
            def l2_panel(j, psum, epp, sbp):
                o = sbp.tile([P, dout], dt, tag="o2")
                if has_b2:
                    nc.vector.tensor_tensor(out=o[:], in0=psum[:],
                                            in1=b2_sb[:],
                                            op=mybir.AluOpType.add)
                    nc.vector.tensor_scalar(out=o[:], in0=o[:], scalar1=0.0,
                                            scalar2=None,
                                            op0=mybir.AluOpType.max)
                else:
                    nc.vector.tensor_scalar(out=o[:], in0=psum[:],
                                            scalar1=0.0, scalar2=None,
                                            op0=mybir.AluOpType.max)
                nc.sync.dma_start(out=out_d[j * P:(j + 1) * P, :],
                                  in_=o[:])

            agg_layer(2, l2_panel, src_lo=zg_lo[:],
                      src_hi=zg_hi[:],
                      ix_lo=i2lo_sb, ix_hi=i2hi_sb)

    nc.compile()
    return nc


# ---------------------------------------------------------------- kernel


def make_in_maps(spec, cores, W1, b1, W2, b2, has_b1, has_b2):
    W1m, W2m = W1.astype(NP_MSG), W2.astype(NP_MSG)
    in_maps = []
    for k in range(NCORES):
        c = cores[k]
        m = dict(xg=c["xg"], ohm=c["ohm"], norm16=c["norm16"],
                 lsidx=c["lsidx"], w1=W1m, w2=W2m,
                 idx2_lo=c["idx2_lo"], idx2_hi=c["idx2_hi"])
        if has_b1:
            m["b1r"] = np.tile(b1, (P, 1)).astype(np.float32)
        if has_b2:
            m["b2r"] = np.tile(b2, (P, 1)).astype(np.float32)
        in_maps.append(m)
    return in_maps


def kernel(x, edge_index, edge_weight, W1, b1, W2, b2):
    x = np.asarray(x, dtype=np.float32)
    W1 = np.asarray(W1, dtype=np.float32)
    W2 = np.asarray(W2, dtype=np.float32)
    b1 = np.asarray(b1, dtype=np.float32)
    b2 = np.asarray(b2, dtype=np.float32)
    n, din = x.shape
    dhid, dout = W1.shape[1], W2.shape[1]

    spec, cores = preprocess(x, edge_index, edge_weight, n)
    has_b1, has_b2 = bool(np.any(b1)), bool(np.any(b2))
    nc = build_program(spec, din, dhid, dout, has_b1, has_b2)
    in_maps = make_in_maps(spec, cores, W1, b1, W2, b2, has_b1, has_b2)

    res = run_bass_kernel_spmd(nc, in_maps, core_ids=list(range(NCORES)))
    # unshard: panel-padded rows -> node order via boundaries
    out = np.empty((n, dout), dtype=np.float32)
    npanel = spec["npanel"]
    shard = spec["shard"]
    for k in range(NCORES):
        r = res.results[k]["out"]
        b = cores[k]["bounds"]
        for j in range(npanel):
            a, e = int(b[j]), int(b[j + 1])
            if e > a:
                out[k * shard + a:k * shard + e] = r[j * P:j * P + (e - a)]
    return out



# revision 33
# speedup vs baseline: 1.0716x; 1.0716x over previous
"""2-layer GCN (GCNConv x2, relu) on 8 Trainium2 NeuronCores.

Strategy (dest-shard, aggregate in 128-dim space, host-built one-hots):
  out1 = relu((A1*x) @ W1 + b1)         [A@(x@W1) == (A@x)@W1]
  z    = out1 @ W2 -> zg, allgathered
  out2 = relu(A1*zg + b2)
where A1 = Dc*(Mw + I)*Dr with the FULL normalization norm_e =
dinv[row]*w*dinv[col] folded into host-precomputed one-hot tiles
oh[e, d] = norm_e * (d == localdest_e), shared by both layers.

Per core: 1/8 of destination nodes. Edges sorted by dest panel (128
dests), chunked into groups of 128. L1 sources are HOST-pregathered
into a sequential stream xg (no device gather). L2 sources are
dma_gather'ed from the allgathered zg (int16 idx, split in 2 halves
< 32768 rows). Per chunk one matmul accumulates into the panel PSUM.
"""
import sys
import numpy as np

sys.path.insert(0, "/opt/trn_rl_repo")

import concourse.bass as bass  # noqa: F401
import concourse.bacc as bacc
import concourse.mybir as mybir
import concourse.tile as tile
from concourse.bass_utils import run_bass_kernel_spmd
from concourse.masks import make_identity

P = 128
NCORES = 8
G = 32    # chunks per dma_gather wave (L2)
SW = 32     # chunks per xg stream DMA wave
SW_OH = 8  # chunks per L2 one-hot stream DMA wave
LS_G = 8  # chunks per local_scatter one-hot build (L1)

F32 = mybir.dt.float32
F16 = mybir.dt.float16
I16 = mybir.dt.int16
MSG_DT = F16  # dtype for messages / dense matmuls
NP_MSG = np.float16


# ---------------------------------------------------------------- CPU prep


def _pack_idx(idx_flat):
    """int16 indices -> [128, ceil(n/16)] wrapped + 8x replicated layout."""
    n = len(idx_flat)
    n16 = -(-n // 16)
    buf = np.zeros(16 * n16, dtype=np.int16)
    buf[:n] = idx_flat
    blk = buf.reshape(n16, 16).T  # idx j at [j%16, j//16]
    return np.tile(blk, (8, 1)).copy()


def preprocess(x, edge_index, edge_weight, n):
    """Per-core inputs + universal chunk grid.

    Dests are grouped into panels of <=128 CONSECUTIVE dests with
    adaptive boundaries chosen per core to fill chunk capacities; the
    per-panel chunk counts (klo_j, khi_j) are universal (max over
    cores). Outputs and zg are written panel-padded (128 rows/panel);
    layer-2 gather indices use AG positions in the padded layout.
    """
    row = np.asarray(edge_index[0], dtype=np.int64)
    col = np.asarray(edge_index[1], dtype=np.int64)
    w = np.asarray(edge_weight, dtype=np.float32)
    shard = n // NCORES
    half = (n + 1) // 2

    # ---- full normalization on host: deg incl self loops, norm per edge
    deg = np.zeros(n, np.float64)
    np.add.at(deg, col, w.astype(np.float64))
    deg += 1.0  # self loop w=1
    dinv = 1.0 / np.sqrt(deg)
    norm = (dinv[row] * w * dinv[col]).astype(np.float32)

    core_of = col // shard

    # ---- per-dest lo/hi in-degree (incl self edge) per core
    # Stream split = source's PANEL-half (enables split AllGather); for
    # boundary construction approximate it by shard-local half.
    hs = shard // 2
    deg_lo = np.zeros(n, np.int64)
    deg_hi = np.zeros(n, np.int64)
    np.add.at(deg_lo, col[(row % shard) < hs], 1)
    np.add.at(deg_hi, col[(row % shard) >= hs], 1)
    selfs_all = np.arange(n)
    deg_lo[(selfs_all % shard) < hs] += 1
    deg_hi[(selfs_all % shard) >= hs] += 1

    # ---- adaptive consecutive panel boundaries per core
    CAP = 7 * P  # max edges per (panel, stream)
    blist = []
    for k in range(NCORES):
        lo_c = deg_lo[k * shard:(k + 1) * shard]
        hi_c = deg_hi[k * shard:(k + 1) * shard]
        b = [0]
        cl = ch = cd = 0
        for ld in range(shard):
            if cd == P or cl + lo_c[ld] > CAP or ch + hi_c[ld] > CAP:
                b.append(ld)
                cl = ch = cd = 0
            cl += lo_c[ld]
            ch += hi_c[ld]
            cd += 1
        b.append(shard)
        blist.append(b)
    npanel = max(len(b) - 1 for b in blist)
    bounds = np.zeros((NCORES, npanel + 1), np.int64)
    for k in range(NCORES):
        b = blist[k]
        while len(b) < npanel + 1:
            b.append(shard)
        bounds[k] = b

    # ---- AG position map (half-major: all cores' first-j0 panels, then
    # the rest) + EXACT stream split by source's panel-half
    j0 = npanel // 2
    off1 = NCORES * j0 * P
    agpos = np.zeros(n, np.int64)
    for k in range(NCORES):
        for j in range(npanel):
            a, b2 = bounds[k, j], bounds[k, j + 1]
            if b2 > a:
                if j < j0:
                    base = k * j0 * P + j * P
                else:
                    base = off1 + k * (npanel - j0) * P + (j - j0) * P
                agpos[k * shard + a:k * shard + b2] = \
                    base + np.arange(b2 - a)
    hi_src = agpos >= off1
    h2 = off1
    assert h2 < 32768 and (NCORES * npanel * P - h2) < 32768

    # exact per-dest stream in-degrees for the universal grid
    deg_lo = np.zeros(n, np.int64)
    deg_hi = np.zeros(n, np.int64)
    np.add.at(deg_lo, col[~hi_src[row]], 1)
    np.add.at(deg_hi, col[hi_src[row]], 1)
    deg_lo[~hi_src] += 1  # self loops
    deg_hi[hi_src] += 1

    # ---- per (core, panel, stream) edge counts -> universal grid
    cnts = np.zeros((NCORES, npanel, 2), np.int64)
    for k in range(NCORES):
        for j in range(npanel):
            a, b2 = bounds[k, j], bounds[k, j + 1]
            cnts[k, j, 0] = deg_lo[k * shard + a:k * shard + b2].sum()
            cnts[k, j, 1] = deg_hi[k * shard + a:k * shard + b2].sum()
    kcnt = np.maximum(1, -(-cnts.max(axis=0) // P))  # [npanel, 2]
    klo, khi = kcnt[:, 0], kcnt[:, 1]
    nlo_ch, nhi_ch = int(klo.sum()), int(khi.sum())
    nchunks = nlo_ch + nhi_ch

    chunks = []
    lo_pos = hi_pos = 0
    for j in range(npanel):
        nj = int(klo[j]) + int(khi[j])
        for i in range(int(klo[j])):
            chunks.append(dict(stream=0, pos=lo_pos, panel=j,
                               first=(i == 0), last=(i == nj - 1)))
            lo_pos += 1
        for i in range(int(khi[j])):
            chunks.append(dict(stream=1, pos=hi_pos, panel=j,
                               first=False, last=(int(klo[j]) + i == nj - 1)))
            hi_pos += 1

    x16 = np.ascontiguousarray(np.asarray(x, dtype=NP_MSG))

    # ---- per-core chunk slot data
    cores = []
    for k in range(NCORES):
        m = core_of == k
        selfs = np.arange(k * shard, (k + 1) * shard, dtype=np.int64)
        r_k = np.concatenate([row[m], selfs])
        c_k = np.concatenate([col[m], selfs])
        nm_k = np.concatenate([norm[m],
                               (dinv[selfs] * dinv[selfs]).astype(np.float32)])
        ld = c_k - k * shard
        panel = np.searchsorted(bounds[k], ld, side="right") - 1
        q = ld - bounds[k][panel]  # slot within panel
        hi = hi_src[r_k].astype(np.int64)
        order = np.lexsort((hi, panel))
        r_k, nm_k, panel, hi, q = (r_k[order], nm_k[order], panel[order],
                                   hi[order], q[order])
        key = panel * 2 + hi
        cnt = np.bincount(key, minlength=npanel * 2)
        goff = np.r_[0, np.cumsum(cnt)]
        src_mat = np.zeros((nchunks, P), np.int64)     # source node per slot
        idx2_lo = np.zeros(nlo_ch * P, np.int64)
        idx2_hi = np.zeros(nhi_ch * P, np.int64)
        lc_mat = np.zeros((nchunks, P), np.int64)      # local dest per slot
        nm_mat = np.zeros((nchunks, P), np.float32)    # norm per slot
        win = np.zeros(npanel * 2, np.int64)
        ag_r = agpos[r_k]
        for ci, c in enumerate(chunks):
            j, st = c["panel"], c["stream"]
            g = j * 2 + st
            a = goff[g] + win[g] * P
            b2 = min(goff[g] + win[g] * P + P, goff[g + 1])
            win[g] += 1
            m2 = max(0, b2 - a)
            if m2 > 0:
                pos = c["pos"] * P
                if st == 0:
                    idx2_lo[pos:pos + m2] = ag_r[a:b2]
                else:
                    idx2_hi[pos:pos + m2] = ag_r[a:b2] - h2
                src_mat[ci, :m2] = r_k[a:b2]
                lc_mat[ci, :m2] = q[a:b2]
                nm_mat[ci, :m2] = nm_k[a:b2]

        # host-pregathered L1 source stream: [128 e, nchunks, 128 f]
        xg = np.ascontiguousarray(
            x16[src_mat].transpose(1, 0, 2))
        # L2 one-hot tiles streamed from HBM: [128 e, nchunks, 128 d]
        ohm = np.zeros((nchunks, P, P), NP_MSG)
        cii, eii = np.nonzero(nm_mat)
        ohm[cii, eii, lc_mat[cii, eii]] = nm_mat[cii, eii].astype(NP_MSG)
        ohm = np.ascontiguousarray(ohm.transpose(1, 0, 2))
        # compact one-hot descriptors for L1's on-device local_scatter build
        ncpad = -(-nchunks // LS_G) * LS_G
        norm16 = np.zeros((P, ncpad), NP_MSG)
        norm16[:, :nchunks] = nm_mat.T.astype(NP_MSG)
        ci_grid = np.broadcast_to(np.arange(nchunks)[:, None], (nchunks, P))
        lsidx = np.full((P, ncpad), -1, np.int16)
        lsidx[:, :nchunks] = np.where(
            nm_mat != 0, (ci_grid % LS_G) * P + lc_mat, -1).T.astype(np.int16)
        cores.append(dict(idx2_lo=_pack_idx(idx2_lo.astype(np.int16)),
                          idx2_hi=_pack_idx(idx2_hi.astype(np.int16)),
                          xg=xg, ohm=ohm, norm16=norm16, lsidx=lsidx,
                          bounds=bounds[k].copy()))

    spec = dict(n=n, shard=shard, npanel=npanel, half=half, h2=h2, j0=j0,
                chunks=chunks, nlo=nlo_ch, nhi=nhi_ch, nchunks=nchunks,
                ncpad=ncpad)
    return spec, cores


# ---------------------------------------------------------------- program


def build_program(spec, din, dhid, dout, has_b1, has_b2):
    n, half = spec["n"], spec["half"]
    npanel, shard = spec["npanel"], spec["shard"]
    h2, j0 = spec["h2"], spec["j0"]
    nrows_pad = npanel * P  # padded rows per core (zg_shard / out)
    nfull = NCORES * nrows_pad
    chunks, nlo, nhi = spec["chunks"], spec["nlo"], spec["nhi"]
    nchunks = spec["nchunks"]
    assert din == P and dout == P and dhid == 2 * P

    nc = bacc.Bacc("TRN2", target_bir_lowering=False, debug=False,
                   num_devices=NCORES, num_swdge_queues=4)
    dt = F32
    ncpad = spec["ncpad"]
    xg_d = nc.dram_tensor("xg", [P, nchunks, din], MSG_DT,
                          kind="ExternalInput")
    oh_d = nc.dram_tensor("ohm", [P, nchunks, P], MSG_DT,
                          kind="ExternalInput")
    norm16_d = nc.dram_tensor("norm16", [P, ncpad], MSG_DT,
                              kind="ExternalInput")
    lsidx_d = nc.dram_tensor("lsidx", [P, ncpad], I16, kind="ExternalInput")
    w1_d = nc.dram_tensor("w1", [din, dhid], MSG_DT, kind="ExternalInput")
    w2_d = nc.dram_tensor("w2", [dhid, dout], MSG_DT, kind="ExternalInput")
    i2lo_d = nc.dram_tensor("idx2_lo", [P, nlo * 8], I16,
                            kind="ExternalInput")
    i2hi_d = nc.dram_tensor("idx2_hi", [P, nhi * 8], I16,
                            kind="ExternalInput")
    b1_d = b2_d = None
    if has_b1:
        b1_d = nc.dram_tensor("b1r", [P, dhid], dt, kind="ExternalInput")
    if has_b2:
        b2_d = nc.dram_tensor("b2r", [P, dout], dt, kind="ExternalInput")
    out_d = nc.dram_tensor("out", [nrows_pad, dout], dt,
                           kind="ExternalOutput")

    with tile.TileContext(nc) as tc:
        with (
            tc.tile_pool(name="const", bufs=1) as cpool,
            tc.tile_pool(name="dram", bufs=1, space="DRAM") as dram,
        ):
            zg_shard = dram.tile([nrows_pad, dout], MSG_DT)
            zg_lo = dram.tile([h2, dout], MSG_DT, addr_space="Shared")
            zg_hi = dram.tile([nfull - h2, dout], MSG_DT,
                              addr_space="Shared")

            ident = cpool.tile([P, P], MSG_DT, tag="ident")
            make_identity(nc, ident[:])
            w1_sb = cpool.tile([din, dhid], MSG_DT, tag="w1")
            nc.sync.dma_start(out=w1_sb[:], in_=w1_d[:])
            w2a_sb = cpool.tile([P, dout], MSG_DT, tag="w2a")
            nc.sync.dma_start(out=w2a_sb[:], in_=w2_d[0:P, :])
            w2b_sb = cpool.tile([P, dout], MSG_DT, tag="w2b")
            nc.sync.dma_start(out=w2b_sb[:], in_=w2_d[P:2 * P, :])
            i2lo_sb = cpool.tile([P, nlo * 8], I16, tag="i2lo")
            nc.sync.dma_start(out=i2lo_sb[:], in_=i2lo_d[:])
            i2hi_sb = cpool.tile([P, nhi * 8], I16, tag="i2hi")
            nc.sync.dma_start(out=i2hi_sb[:], in_=i2hi_d[:])
            norm16_sb = cpool.tile([P, ncpad], MSG_DT, tag="norm16")
            nc.sync.dma_start(out=norm16_sb[:], in_=norm16_d[:])
            lsidx_sb = cpool.tile([P, ncpad], I16, tag="lsidx")
            nc.sync.dma_start(out=lsidx_sb[:], in_=lsidx_d[:])
            b1_sb = b2_sb = None
            if has_b1:
                b1_sb = cpool.tile([P, dhid], dt, tag="b1")
                nc.sync.dma_start(out=b1_sb[:], in_=b1_d[:])
            if has_b2:
                b2_sb = cpool.tile([P, dout], dt, tag="b2")
                nc.sync.dma_start(out=b2_sb[:], in_=b2_d[:])

            # -------- shared chunk machinery ------------------------------
            def agg_layer(layer, emit_panel, src_lo=None, src_hi=None,
                          ix_lo=None, ix_hi=None, after_wave=None):
                """layer 1: stream xg sequentially; layer 2: dma_gather."""
                with (
                    tc.tile_pool(name=f"glo{layer}", bufs=6) as glo,
                    tc.tile_pool(name=f"ghi{layer}", bufs=6) as ghi,
                    tc.tile_pool(name=f"oh{layer}", bufs=4) as ohp,
                    tc.tile_pool(name=f"agg{layer}", bufs=2 if layer == 1
                                 else 6, space="PSUM") as aggp,
                    tc.tile_pool(name=f"ep{layer}", bufs=1,
                                 space="PSUM") as epp,
                    tc.tile_pool(name=f"sb{layer}", bufs=3) as sbp,
                ):
                    wave_t = [None, None]
                    oh_t = None
                    oh_ls = None
                    psum = None
                    gq = [0]  # round-robin SWDGE queue for gather waves
                    wtiles = [{}, {}]
                    # L1 one-hots: local_scatter only BEFORE AG1 fires so the
                    # GpSimd queue is clear for L2 gathers; stream the rest.
                    ci_sw = next((i for i, cc in enumerate(chunks)
                                  if cc["panel"] >= j0), nchunks)

                    def ensure_wave(st2, wv2):
                        if wv2 in wtiles[st2]:
                            return
                        pool = glo if st2 == 0 else ghi
                        idx_sb = ix_lo if st2 == 0 else ix_hi
                        src = src_lo if st2 == 0 else src_hi
                        nw = nlo if st2 == 0 else nhi
                        gsz = min(G, nw - wv2 * G)
                        t = pool.tile([P, G, P], MSG_DT, tag="gw")
                        nc.gpsimd.dma_gather(
                            out_ap=t[:, :gsz, :], in_ap=src,
                            idxs_ap=idx_sb[:, wv2 * G * 8:
                                           wv2 * G * 8 + gsz * 8],
                            num_idxs=gsz * P, num_idxs_reg=gsz * P,
                            elem_size=P, single_packet=False,
                            queue_num=gq[0])
                        gq[0] = (gq[0] + 1) % 4
                        wtiles[st2][wv2] = t

                    for ci, c in enumerate(chunks):
                        st, pos, j = c["stream"], c["pos"], c["panel"]
                        # one-hot built on device from compact (lc, norm)
                        if layer == 1:  # local_scatter on idle GpSimd
                            wv_o, slot_o = divmod(ci, LS_G)
                            if slot_o == 0:
                                oh_ls = ohp.tile([P, LS_G, P], MSG_DT,
                                                 tag="ohls")
                                a = wv_o * LS_G
                                nc.gpsimd.local_scatter(
                                    out_ap=oh_ls[:],
                                    data_ap=norm16_sb[:, a:a + LS_G],
                                    idxs_ap=lsidx_sb[:, a:a + LS_G],
                                    channels=P, num_elems=LS_G * P,
                                    num_idxs=LS_G)
                            oh = oh_ls[:, slot_o, :]
                        else:           # streamed one-hot wave from HBM
                            base = ci_sw if layer == 1 else 0
                            wv_o, slot_o = divmod(ci - base, SW_OH)
                            if slot_o == 0:
                                a = base + wv_o * SW_OH
                                osz = min(SW_OH, nchunks - a)
                                oh_t = ohp.tile([P, SW_OH, P], MSG_DT,
                                                tag="ohw")
                                oeng = nc.scalar if layer == 2 else nc.sync
                                oeng.dma_start(
                                    out=oh_t[:, :osz, :],
                                    in_=oh_d[:, a:a + osz, :])
                            oh = oh_t[:, slot_o, :]
                        # source tile
                        if layer == 1:
                            gt = None  # xg stream, ci-major
                            wv_x, slot_x = divmod(ci, SW)
                            if slot_x == 0:
                                osz = min(SW, nchunks - wv_x * SW)
                                t = glo.tile([P, SW, P], MSG_DT, tag="xw")
                                eng = nc.scalar if wv_x % 2 else nc.sync
                                eng.dma_start(
                                    out=t[:, :osz, :],
                                    in_=xg_d[:, wv_x * SW:wv_x * SW + osz, :])
                                wave_t[0] = t
                            gt = wave_t[0][:, slot_x, :]
                        else:
                            wv, slot = divmod(pos, G)
                            ensure_wave(st, wv)
                            gt = wtiles[st][wv][:, slot, :]
                        if c["first"]:
                            psum = aggp.tile([P, P], F32, space="PSUM",
                                             tag="agg")
                        if layer == 1:  # feat-major: psum[f,d] += g.T @ oh
                            nc.tensor.matmul(out=psum[:], lhsT=gt, rhs=oh,
                                             start=c["first"], stop=c["last"])
                        else:           # dest-major: psum[d,f] += oh.T @ g
                            nc.tensor.matmul(out=psum[:], lhsT=oh, rhs=gt,
                                             start=c["first"], stop=c["last"])
                        if c["last"]:
                            emit_panel(j, psum, epp, sbp)

            # -------- layer 1 ---------------------------------------------
            def l1_panel(j, psum, epp, sbp):
                aggT = sbp.tile([P, P], MSG_DT, tag="aggT")
                nc.vector.tensor_copy(out=aggT[:], in_=psum[:])
                h1p = epp.tile([P, dhid], F32, space="PSUM", tag="h1p")
                nc.tensor.matmul(out=h1p[:], lhsT=aggT[:], rhs=w1_sb[:],
                                 start=True, stop=True)
                h1 = sbp.tile([P, dhid], MSG_DT, tag="h1")
                if has_b1:
                    nc.vector.tensor_tensor(out=h1[:], in0=h1p[:],
                                            in1=b1_sb[:],
                                            op=mybir.AluOpType.add)
                    nc.vector.tensor_scalar(out=h1[:], in0=h1[:],
                                            scalar1=0.0, scalar2=None,
                                            op0=mybir.AluOpType.max)
                else:
                    nc.vector.tensor_scalar(out=h1[:], in0=h1p[:],
                                            scalar1=0.0, scalar2=None,
                                            op0=mybir.AluOpType.max)
                tp0 = epp.tile([P, P], MSG_DT, space="PSUM", tag="tp0")
                nc.tensor.transpose(out=tp0[:], in_=h1[:, 0:P],
                                    identity=ident[:])
                tp1 = epp.tile([P, P], MSG_DT, space="PSUM", tag="tp1")
                nc.tensor.transpose(out=tp1[:], in_=h1[:, P:2 * P],
                                    identity=ident[:])
                h1t0 = sbp.tile([P, P], MSG_DT, tag="h1t0")
                nc.vector.tensor_copy(out=h1t0[:], in_=tp0[:])
                h1t1 = sbp.tile([P, P], MSG_DT, tag="h1t1")
                nc.vector.tensor_copy(out=h1t1[:], in_=tp1[:])
                zp = epp.tile([P, dout], F32, space="PSUM", tag="zp")
                nc.tensor.matmul(out=zp[:], lhsT=h1t0[:], rhs=w2a_sb[:],
                                 start=True, stop=False)
                nc.tensor.matmul(out=zp[:], lhsT=h1t1[:], rhs=w2b_sb[:],
                                 start=False, stop=True)
                zg = sbp.tile([P, dout], MSG_DT, tag="zg")
                nc.vector.tensor_copy(out=zg[:], in_=zp[:])
                nc.sync.dma_start(out=zg_shard[j * P:(j + 1) * P, :],
                                  in_=zg[:])
                if j == j0 - 1:
                    # first-half AllGather overlaps the rest of layer 1
                    nc.gpsimd.collective_compute(
                        "AllGather", mybir.AluOpType.bypass,
                        replica_groups=[list(range(NCORES))],
                        ins=[zg_shard[0:j0 * P, :].opt()],
                        outs=[zg_lo[:].opt()])

            agg_layer(1, l1_panel)

            nc.gpsimd.collective_compute(
                "AllGather", mybir.AluOpType.bypass,
                replica_groups=[list(range(NCORES))],
                ins=[zg_shard[j0 * P:nrows_pad, :].opt()],
                outs=[zg_hi[:].opt()])

            # -------- layer 2 ---------------------------------------------
            def l2_panel(j, psum, epp, sbp):
                o = sbp.tile([P, dout], dt, tag="o2")
                if has_b2:
                    nc.vector.tensor_tensor(out=o[:], in0=psum[:],
                                            in1=b2_sb[:],
                                            op=mybir.AluOpType.add)
                    nc.vector.tensor_scalar(out=o[:], in0=o[:], scalar1=0.0,
                                            scalar2=None,
                                            op0=mybir.AluOpType.max)
                else:
                    nc.vector.tensor_scalar(out=o[:], in0=psum[:],
                                            scalar1=0.0, scalar2=None,
                                            op0=mybir.AluOpType.max)
                nc.sync.dma_start(out=out_d[j * P:(j + 1) * P, :],
                                  in_=o[:])

            agg_layer(2, l2_panel, src_lo=zg_lo[:],
                      src_hi=zg_hi[:],
                      ix_lo=i2lo_sb, ix_hi=i2hi_sb)

    nc.compile()
    return nc


# ---------------------------------------------------------------- kernel


def make_in_maps(spec, cores, W1, b1, W2, b2, has_b1, has_b2):
    W1m, W2m = W1.astype(NP_MSG), W2.astype(NP_MSG)
    in_maps = []
    for k in range(NCORES):
        c = cores[k]
        m = dict(xg=c["xg"], ohm=c["ohm"], norm16=c["norm16"],
                 lsidx=c["lsidx"], w1=W1m, w2=W2m,
                 idx2_lo=c["idx2_lo"], idx2_hi=c["idx2_hi"])
        if has_b1:
            m["b1r"] = np.tile(b1, (P, 1)).astype(np.float32)
        if has_b2:
            m["b2r"] = np.tile(b2, (P, 1)).astype(np.float32)
        in_maps.append(m)
    return in_maps


def kernel(x, edge_index, edge_weight, W1, b1, W2, b2):
    x = np.asarray(x, dtype=np.float32)
    W1 = np.asarray(W1, dtype=np.float32)
    W2 = np.asarray(W2, dtype=np.float32)
    b1 = np.asarray(b1, dtype=np.float32)
    b2 = np.asarray(b2, dtype=np.float32)
    n, din = x.shape
    dhid, dout = W1.shape[1], W2.shape[1]

    spec, cores = preprocess(x, edge_index, edge_weight, n)
    has_b1, has_b2 = bool(np.any(b1)), bool(np.any(b2))
    nc = build_program(spec, din, dhid, dout, has_b1, has_b2)
    in_maps = make_in_maps(spec, cores, W1, b1, W2, b2, has_b1, has_b2)

    res = run_bass_kernel_spmd(nc, in_maps, core_ids=list(range(NCORES)))
    # unshard: panel-padded rows -> node order via boundaries
    out = np.empty((n, dout), dtype=np.float32)
    npanel = spec["npanel"]
    shard = spec["shard"]
    for k in range(NCORES):
        r = res.results[k]["out"]
        b = cores[k]["bounds"]
        for j in range(npanel):
            a, e = int(b[j]), int(b[j + 1])
            if e > a:
                out[k * shard + a:k * shard + e] = r[j * P:j * P + (e - a)]
    return out

